# revision 1
# baseline (speedup 1.0000x reference)
"""Baichuan attention on 8 Trainium2 NeuronCores — tensor-parallel over heads.

Sharding: core c computes heads [4c, 4c+4): its slice of the fused QKV
projection, attention for those heads, then 1/8 of o_proj's output columns
after an AllGather of the per-core context slices (moves 4MB/rank instead of
a 32MB AllReduce of partial sums; mathematically identical to the module's
world_size logic).

Layout: scores are computed transposed (scoresT[k, q] blocks) so the PE
contraction dim always sits on SBUF partitions and every matmul streams a
512-wide moving operand. Matmul operands are fp16 (1 cyc/row on the PE) with
fp32 PSUM accumulation — measured end-to-end error vs the fp32 reference is
~6e-4 absmax-relative, on par with the f32r (tf32) path. The AllGather is
chunked over four s_q blocks so collective latency and o_proj overlap the
attention of later blocks, keeping the PE stream dense (HAM stays warm).
"""

import numpy as np

import concourse.bacc as bacc
import concourse.mybir as mybir
import concourse.tile as tile
from concourse.bass_utils import run_bass_kernel_spmd

F32 = mybir.dt.float32

N_CORES = 8
NUM_HEADS = 32
HEAD_DIM = 128
P = 128          # SBUF partitions / PE contraction tile
SQ = 512         # s_q block width (PSUM bank = 512 fp32)
MM_MODE = "f16"  # 'f16' | 'f32' (operand dtype for matmuls)

_CACHE: dict = {}


def _mm_dtype(mode):
    return {"f16": mybir.dt.float16, "f32": F32}[mode]


def build(S, H, block_cls, mode=MM_MODE):
    """Build the SPMD program. block_cls[(t, b)] = 'plain' | 'mask' for every
    computed scoresT block ([128 s_k] x [SQ s_q]); absent = fully masked, skip.
    """
    MD = _mm_dtype(mode)
    hpc = NUM_HEADS // N_CORES          # heads per core
    dpc = hpc * HEAD_DIM                # per-core slice of the hidden dim
    n_ht = H // P                       # contraction tiles for QKV/o_proj
    n_qk = 2 * dpc // P                 # q+k output tiles
    n_sq = S // SQ                      # s_q blocks
    n_st = S // P                       # s_k tiles
    scale = 1.0 / np.sqrt(np.float32(HEAD_DIM))
    s_half = S // 2
    sb_per_half = s_half // SQ

    nc = bacc.Bacc("TRN2", target_bir_lowering=False, debug=False,
                   num_devices=N_CORES)

    xT = nc.dram_tensor("xT", [H, S], MD, kind="ExternalInput")
    wqkT = nc.dram_tensor("wqkT", [H, 2 * dpc], MD, kind="ExternalInput")
    wvT = nc.dram_tensor("wvT", [H, dpc], MD, kind="ExternalInput")
    maskT = nc.dram_tensor("maskT", [S, S], F32, kind="ExternalInput")
    woT = nc.dram_tensor("woT", [H, dpc], MD, kind="ExternalInput")
    out_cols = nc.dram_tensor("out_cols", [S, dpc], F32, kind="ExternalOutput")

    # AllGather in head-pair chunks: gat[b][pp] holds local heads
    # {2pp, 2pp+1} for s_q block b; ct[b][pp] gathers those pairs from all
    # ranks. o_proj consumes them against host-permuted w_o rows.
    gat_b = [[nc.dram_tensor(f"gat_{b}_{pp}", [dpc // 2, SQ], MD)
              for pp in range(2)] for b in range(n_sq)]
    ct_b = [[nc.dram_tensor(f"ct_{b}_{pp}", [H // 2, SQ], MD,
                            addr_space="Shared") for pp in range(2)]
            for b in range(n_sq)]

    xT_t = xT.ap().rearrange("(t p) s -> p t s", p=P)
    wqkT_t = wqkT.ap().rearrange("(t p) o -> p t o", p=P)
    wvT_t = wvT.ap().rearrange("(t p) o -> p t o", p=P)
    woT_t = woT.ap().rearrange("(t p) j -> p t j", p=P)

    # sorted by (b, t) so block b=0's diagonal tiles arrive first
    mask_blocks = sorted({k for k, v in block_cls.items() if v == "mask"},
                         key=lambda k: (k[1], k[0]))
    mask_slot = {k: i for i, k in enumerate(mask_blocks)}

    with tile.TileContext(nc) as tc:
        with (
            tc.tile_pool(name="consts", bufs=1) as cpool,
            tc.tile_pool(name="span", bufs=1) as span,
        ):
            ones_f = cpool.tile([P, P], F32, tag="ones_f")
            nc.gpsimd.memset(ones_f[:], 1.0)
            ones_sq = cpool.tile([P, P], MD, tag="ones_sq")
            nc.scalar.copy(ones_sq[:], ones_f[:])

            # v ([s_k, d] natural, all heads) and q/k (transposed, all heads)
            # live in SBUF across phases 1-2; QKV evictions write them
            # directly (no DRAM bounce)
            v_sb = span.tile([P, n_st, dpc], MD, tag="v")
            qk_all = span.tile([P, n_qk, S], MD, tag="qk")

            # =============== phase 1: QKV projection ===============
            # q/k in transposed orientation -> DRAM scratch; v in natural
            # orientation (x stationary, Wv moving) -> resident v_sb.
            with (
                tc.tile_pool(name="qkv_x", bufs=1) as xpool,
                tc.tile_pool(name="qkv_w", bufs=3) as wpool,
                tc.tile_pool(name="qkv_wv", bufs=1) as wvpool,
                tc.tile_pool(name="qkv_ps", bufs=4, space="PSUM") as pspool,
            ):
                wv_sb = wvpool.tile([P, n_ht, dpc], MD, tag="wv")
                for half in range(2):
                    # first q/k weight tile ahead of the bulk x transfer so
                    # the PE starts within a few us; wv's 4MB comes after the
                    # second w tile. x arrives as two SQ-wide quarters so the
                    # next half's load overlaps this half's tail.
                    w_tiles = {}
                    w_tiles[0] = wpool.tile([P, n_ht, P], MD, tag="w",
                                            name="w_tile")
                    nc.sync.dma_start(
                        w_tiles[0][:], wqkT_t[:, :, 0 * P:1 * P])
                    xq = []
                    for sb in range(sb_per_half):
                        x_tile = xpool.tile([P, n_ht, SQ], MD, tag="x",
                                            bufs=3, name="x_tile")
                        lo = half * s_half + sb * SQ
                        for t in range(n_ht):
                            nc.sync.dma_start(
                                x_tile[:, t, :], xT_t[:, t, lo:lo + SQ])
                        xq.append(x_tile)
                    for ot in range(n_qk):
                        if ot not in w_tiles:
                            w_tiles[ot] = wpool.tile([P, n_ht, P], MD,
                                                     tag="w", name="w_tile")
                            nc.sync.dma_start(
                                w_tiles[ot][:],
                                wqkT_t[:, :, ot * P:(ot + 1) * P])
                        w_tile = w_tiles[ot]
                        if half == 0 and ot == 1:
                            nc.sync.dma_start(wv_sb[:], wvT_t[:])
                        for sb in range(sb_per_half):
                            ps = pspool.tile([P, SQ], F32, tag="qkv")
                            for t in range(n_ht):
                                nc.tensor.matmul(
                                    ps[:],
                                    w_tile[:, t, :],
                                    xq[sb][:, t, :],
                                    start=(t == 0), stop=(t == n_ht - 1))
                            # fold the softmax scale into q at eviction;
                            # write straight into the resident qk tile
                            mul = scale if ot < dpc // P else 1.0
                            lo = half * s_half + sb * SQ
                            nc.scalar.mul(qk_all[:, ot, lo:lo + SQ],
                                          ps[:], mul)
                    # v: psum [s=128, dpc] accumulated over h-tiles
                    for sti in range(s_half // P):
                        st_g = half * (s_half // P) + sti
                        sb, off = (sti * P) // SQ, (sti * P) % SQ
                        ps_v = pspool.tile([P, dpc], F32, tag="qkv")
                        for t in range(n_ht):
                            nc.tensor.matmul(
                                ps_v[:],
                                xq[sb][:, t, off:off + P],
                                wv_sb[:, t, :],
                                start=(t == 0), stop=(t == n_ht - 1))
                        nc.vector.tensor_copy(v_sb[:, st_g, :], ps_v[:])

            # ====== phases 2-4: attention / chunked AllGather / o_proj ======
            with (
                tc.tile_pool(name="at_mask", bufs=1) as mpool,
                tc.tile_pool(name="at_exp", bufs=4) as epool,
                tc.tile_pool(name="at_out", bufs=3) as opool,
                tc.tile_pool(name="at_r", bufs=2) as rpool,
                tc.tile_pool(name="op_w", bufs=1) as owpool,
                tc.tile_pool(name="op_ct", bufs=40) as ctpool,
                tc.tile_pool(name="op_stage", bufs=4) as ospool,
                tc.tile_pool(name="at_ps", bufs=2, space="PSUM") as aps,
                tc.tile_pool(name="op_ps", bufs=2, space="PSUM") as opspool,
            ):
                if mask_blocks:
                    mtile = mpool.tile([P, len(mask_blocks), SQ], F32,
                                       tag="mask")
                    for (t, b), i in mask_slot.items():
                        nc.sync.dma_start(
                            mtile[:, i, :],
                            maskT.ap()[t * P:(t + 1) * P,
                                       b * SQ:(b + 1) * SQ])
                wo_sb = owpool.tile([P, n_ht, dpc], MD, tag="wo")

                def emit_oproj(b, cts):
                    for st in range(SQ // P):
                        ps = opspool.tile([P, dpc], F32, tag="op",
                                          name="op_ps")
                        for t in range(n_ht):
                            nc.tensor.matmul(
                                ps[:],
                                cts[t][:, st * P:(st + 1) * P],
                                wo_sb[:, t, :],
                                start=(t == 0), stop=(t == n_ht - 1))
                        ob = ospool.tile([P, dpc], F32, tag="ostage",
                                         name="ostage")
                        nc.scalar.copy(ob[:], ps[:])
                        nc.sync.dma_start(
                            out_cols.ap()[b * SQ + st * P:
                                          b * SQ + (st + 1) * P, :], ob[:])

                pending = []
                for b in range(n_sq):
                    ts_here = [t for t in range(n_st) if (t, b) in block_cls]
                    assert ts_here, "fully-masked s_q block unsupported"
                    for h in range(hpc):
                        q_sl = qk_all[:, h, b * SQ:(b + 1) * SQ]
                        ps_o = aps.tile([P, SQ], F32, tag="out",
                                        name="ps_o")
                        ps_row = aps.tile([P, SQ], F32, tag="row",
                                          name="ps_row")
                        for i, t in enumerate(ts_here):
                            ps_s = aps.tile([P, SQ], F32, tag="scores",
                                            name="ps_s")
                            nc.tensor.matmul(
                                ps_s[:],
                                qk_all[:, hpc + h, t * P:(t + 1) * P],
                                q_sl, start=True, stop=True)
                            if block_cls[(t, b)] == "mask":
                                nc.vector.tensor_add(
                                    ps_s[:], ps_s[:],
                                    mtile[:, mask_slot[(t, b)], :])
                            ex = epool.tile([P, SQ], MD, tag="exp",
                                            name="ex")
                            nc.scalar.activation(
                                ex[:], ps_s[:],
                                mybir.ActivationFunctionType.Exp)
                            first, last = i == 0, i == len(ts_here) - 1
                            nc.tensor.matmul(
                                ps_o[:], v_sb[:, t, h * P:(h + 1) * P],
                                ex[:], start=first, stop=last)
                            # rowsum broadcast to all partitions via the
                            # all-ones stationary operand
                            nc.tensor.matmul(
                                ps_row[:], ones_sq[:], ex[:],
                                start=first, stop=last)
                        # evict ps_o to SBUF at once so the psum bank frees
                        # without waiting for the (slow) reciprocal
                        onum = rpool.tile([P, SQ], F32, tag="onum",
                                          name="onum")
                        nc.vector.tensor_copy(onum[:], ps_o[:])
                        recip = rpool.tile([P, SQ], F32, tag="recip",
                                           name="recip")
                        nc.vector.reciprocal(recip[:], ps_row[:])
                        ob = opool.tile([P, SQ], MD, tag="ob", name="ob")
                        nc.vector.tensor_mul(ob[:], onum[:], recip[:])
                        nc.sync.dma_start(
                            gat_b[b][h // 2].ap()[(h % 2) * P:
                                                  (h % 2 + 1) * P, :], ob[:])
                        if h % 2 == 1:
                            nc.gpsimd.collective_compute(
                                "AllGather", mybir.AluOpType.bypass,
                                replica_groups=[list(range(N_CORES))],
                                ins=[gat_b[b][h // 2].ap().opt()],
                                outs=[ct_b[b][h // 2].ap().opt()])

                    if b == 0:
                        # wo arrives during the first AllGather, off the
                        # startup critical path
                        nc.sync.dma_start(wo_sb[:], woT_t[:])

                    # prefetch this block's gathered context tiles; k-tile t
                    # of the half-gathers pairs with the host-permuted w_o
                    # row block t
                    cts = []
                    for pp in range(2):
                        ct_t = ct_b[b][pp].ap().rearrange(
                            "(t p) s -> p t s", p=P)
                        for t in range(n_ht // 2):
                            c_t = ctpool.tile([P, SQ], MD, tag="ct")
                            nc.sync.dma_start(c_t[:], ct_t[:, t, :])
                            cts.append(c_t)
                    # o_proj for block b is emitted after attention b+1 so
                    # the PE prefers attention work and o_proj acts as filler
                    pending.append((b, cts))
                    if len(pending) == 2:
                        emit_oproj(*pending.pop(0))
                for bb, ccts in pending:
                    emit_oproj(bb, ccts)

    nc.compile()
    return nc


def _classify_blocks(maskT_np, S):
    """Classify each [128, SQ] scoresT block of the (transposed) mask."""
    cls = {}
    for t in range(S // P):
        rows = maskT_np[t * P:(t + 1) * P]
        for b in range(S // SQ):
            blk = rows[:, b * SQ:(b + 1) * SQ]
            if np.all(blk <= -1e30):
                continue                      # fully masked: skip compute
            if np.all(blk == 0.0):
                cls[(t, b)] = "plain"
            else:
                cls[(t, b)] = "mask"
    return cls


def make_in_maps(hidden_states, attention_mask, w_pack, w_o):
    B, S, H = hidden_states.shape
    hpc = NUM_HEADS // N_CORES
    dpc = hpc * HEAD_DIM
    np_md = mybir.dt.np(_mm_dtype(MM_MODE))
    xT = np.ascontiguousarray(hidden_states[0].T).astype(np_md)
    maskT_np = np.ascontiguousarray(
        np.broadcast_to(attention_mask, (1, 1, S, S))[0, 0].T,
        dtype=np.float32)
    # w_o rows permuted to match the head-pair AllGather layout:
    # [pp][rank][head-in-pair] blocks of 128
    perm = np.concatenate(
        [np.arange(128 * (4 * r + 2 * pp + hh),
                   128 * (4 * r + 2 * pp + hh) + 128)
         for pp in (0, 1) for r in range(N_CORES) for hh in (0, 1)])
    in_maps = []
    for c in range(N_CORES):
        sl = slice(c * dpc, (c + 1) * dpc)
        wqk_c = np.concatenate(
            [w_pack[0 * H:1 * H][sl], w_pack[1 * H:2 * H][sl]], axis=0)
        woT_c = np.ascontiguousarray(w_o[sl].T)[perm]
        in_maps.append({
            "xT": xT,
            "wqkT": np.ascontiguousarray(wqk_c.T).astype(np_md),
            "wvT": np.ascontiguousarray(w_pack[2 * H:3 * H][sl].T
                                        ).astype(np_md),
            "maskT": maskT_np,
            "woT": np.ascontiguousarray(woT_c).astype(np_md),
        })
    return in_maps, maskT_np


def kernel(hidden_states, attention_mask, w_pack, w_o):
    B, S, H = hidden_states.shape
    assert B == 1 and H == NUM_HEADS * HEAD_DIM
    assert S % (2 * SQ) == 0

    in_maps, maskT_np = make_in_maps(hidden_states, attention_mask,
                                     w_pack, w_o)
    block_cls = _classify_blocks(maskT_np, S)

    key = (S, H, tuple(sorted(block_cls.items())), MM_MODE)
    if key not in _CACHE:
        _CACHE[key] = build(S, H, block_cls, MM_MODE)
    nc = _CACHE[key]

    res = run_bass_kernel_spmd(nc, in_maps, core_ids=list(range(N_CORES)))
    out = np.concatenate(
        [res.results[c]["out_cols"] for c in range(N_CORES)], axis=1)
    return out.reshape(1, S, H).astype(np.float32)



# revision 9
# speedup vs baseline: 1.0117x; 1.0117x over previous
"""Baichuan attention on 8 Trainium2 NeuronCores — tensor-parallel over heads.

Sharding: core c computes heads [4c, 4c+4): its slice of the fused QKV
projection, attention for those heads, then 1/8 of o_proj's output columns
after an AllGather of the per-core context slices (moves 4MB/rank instead of
a 32MB AllReduce of partial sums; mathematically identical to the module's
world_size logic).

Layout: scores are computed transposed (scoresT[k, q] blocks) so the PE
contraction dim always sits on SBUF partitions and every matmul streams a
512-wide moving operand. Matmul operands are fp16 (1 cyc/row on the PE) with
fp32 PSUM accumulation.

v2 pipeline notes (from the v1 trace: PE 87% busy, 102us of gaps):
- scores land in [128, 2x512] two-bank PSUM tiles so ONE exp activation
  covers a k-tile pair: halves the Scalar engine's per-tile overhead, which
  otherwise paces the PE in the attention phase.
- the causal mask is DMA'd straight from DRAM into the PSUM banks and the
  scores matmul runs with start=False (accumulate onto the mask): the DVE
  mask-adds disappear.
- softmax denominators use reciprocal_approx_fast (1 DVE op, ~18 good bits)
  instead of reciprocal (4us each); the normalize multiply reads the PSUM
  accumulator directly. PSUM banks now recycle ~1us after the last matmul,
  which removes the 3-4us PE stalls v1 had on every head.
- o_proj is emitted as 8 half-contraction chains per block, two after each
  head's attention, split by AllGather half so a chain only ever consumes a
  collective that finished a full block earlier. PSUM: scores 2x2 banks,
  ps_o 2, ps_row 1, o_proj 1 = 8.
- DMAs are spread across engine queues (x: sync+scalar, weights: vector,
  gathers/out: scalar, mask preloads: vector) and the first weight/x tiles
  are split so the PE starts ~1.5us into the kernel instead of 16us.
"""

import numpy as np

import concourse.bacc as bacc
import concourse.mybir as mybir
import concourse.tile as tile
from concourse.bass_utils import run_bass_kernel_spmd

F32 = mybir.dt.float32

N_CORES = 8
NUM_HEADS = 32
HEAD_DIM = 128
P = 128          # SBUF partitions / PE contraction tile
SQ = 512         # s_q block width (PSUM bank = 512 fp32)
MM_MODE = "f16"  # 'f16' | 'f32' (operand dtype for matmuls)

_CACHE: dict = {}


def _mm_dtype(mode):
    return {"f16": mybir.dt.float16, "f32": F32}[mode]


def build(S, H, block_cls, mode=MM_MODE):
    """Build the SPMD program. block_cls[(t, b)] = 'plain' | 'mask' for every
    computed scoresT block ([128 s_k] x [SQ s_q]); absent = fully masked, skip.
    """
    MD = _mm_dtype(mode)
    hpc = NUM_HEADS // N_CORES          # heads per core
    dpc = hpc * HEAD_DIM                # per-core slice of the hidden dim
    n_ht = H // P                       # contraction tiles for QKV/o_proj
    n_qk = 2 * dpc // P                 # q+k output tiles
    n_sq = S // SQ                      # s_q blocks
    n_st = S // P                       # s_k tiles
    scale = 1.0 / np.sqrt(np.float32(HEAD_DIM))
    s_half = S // 2
    sb_per_half = s_half // SQ
    n_hh = n_ht // 2                    # o_proj k-tiles per gather half

    nc = bacc.Bacc("TRN2", target_bir_lowering=False, debug=False,
                   num_devices=N_CORES)

    xT = nc.dram_tensor("xT", [H, S], MD, kind="ExternalInput")
    wqkT = nc.dram_tensor("wqkT", [H, 2 * dpc], MD, kind="ExternalInput")
    wvT = nc.dram_tensor("wvT", [H, dpc], MD, kind="ExternalInput")
    maskT = nc.dram_tensor("maskT", [S, S], F32, kind="ExternalInput")
    woT = nc.dram_tensor("woT", [H, dpc], MD, kind="ExternalInput")
    out_cols = nc.dram_tensor("out_cols", [S, dpc], F32, kind="ExternalOutput")

    # AllGather in head-pair chunks: gat[b][pp] holds local heads
    # {2pp, 2pp+1} for s_q block b; ct[b][pp] gathers those pairs from all
    # ranks. o_proj consumes them against host-permuted w_o rows.
    gat_b = [[nc.dram_tensor(f"gat_{b}_{pp}", [dpc // 2, SQ], MD)
              for pp in range(2)] for b in range(n_sq)]
    ct_b = [[nc.dram_tensor(f"ct_{b}_{pp}", [H // 2, SQ], MD,
                            addr_space="Shared") for pp in range(2)]
            for b in range(n_sq)]

    xT_t = xT.ap().rearrange("(t p) s -> p t s", p=P)
    wqkT_t = wqkT.ap().rearrange("(t p) o -> p t o", p=P)
    wvT_t = wvT.ap().rearrange("(t p) o -> p t o", p=P)
    woT_t = woT.ap().rearrange("(t p) j -> p t j", p=P)

    with tile.TileContext(nc) as tc:
        with (
            tc.tile_pool(name="consts", bufs=1) as cpool,
            tc.tile_pool(name="span", bufs=1) as span,
        ):
            ones_f = cpool.tile([P, P], F32, tag="ones_f")
            nc.gpsimd.memset(ones_f[:], 1.0)
            ones_sq = cpool.tile([P, P], MD, tag="ones_sq")
            nc.scalar.copy(ones_sq[:], ones_f[:])

            # v ([s_k, d] natural, all heads) and q/k (transposed, all heads)
            # live in SBUF across phases 1-2; QKV evictions write them
            # directly (no DRAM bounce)
            v_sb = span.tile([P, n_st, dpc], MD, tag="v")
            qk_all = span.tile([P, n_qk, S], MD, tag="qk")

            # =============== phase 1: QKV projection ===============
            # q/k in transposed orientation -> resident qk_all; v in natural
            # orientation (x stationary, Wv moving) -> resident v_sb.
            with (
                tc.tile_pool(name="qkv_x", bufs=1) as xpool,
                tc.tile_pool(name="qkv_w", bufs=3) as wpool,
                tc.tile_pool(name="qkv_wv", bufs=1) as wvpool,
                tc.tile_pool(name="qkv_ps", bufs=4, space="PSUM") as pspool,
            ):
                wv_sb = wvpool.tile([P, n_ht, dpc], MD, tag="wv")
                for half in range(2):
                    # first q/k weight tile split into quarters (vector
                    # queue) and the first x quarter split per-t across two
                    # queues so the first chain starts ~1.5us in; later
                    # tiles ship as single 3D DMAs.
                    w_tiles = {}
                    w_tiles[0] = wpool.tile([P, n_ht, P], MD, tag="w",
                                            name="w_tile")
                    if half == 0:
                        for wq in range(4):
                            nc.gpsimd.dma_start(
                                w_tiles[0][:, 8 * wq:8 * (wq + 1), :],
                                wqkT_t[:, 8 * wq:8 * (wq + 1), 0:P])
                    else:
                        nc.gpsimd.dma_start(
                            w_tiles[0][:], wqkT_t[:, :, 0:P])
                    xq = []
                    for sb in range(sb_per_half):
                        x_tile = xpool.tile([P, n_ht, SQ], MD, tag="x",
                                            bufs=3, name="x_tile")
                        lo = half * s_half + sb * SQ
                        if half == 0 and sb == 0:
                            # fine-grained start: t0-7 individually on
                            # alternating queues, bulk tails per queue
                            for t in range(8):
                                eng = nc.sync if t % 2 == 0 else nc.scalar
                                eng.dma_start(
                                    x_tile[:, t, :], xT_t[:, t, lo:lo + SQ])
                            nc.sync.dma_start(
                                x_tile[:, 8:20, :], xT_t[:, 8:20, lo:lo + SQ])
                            nc.scalar.dma_start(
                                x_tile[:, 20:, :], xT_t[:, 20:, lo:lo + SQ])
                        else:
                            eng = nc.sync if sb % 2 == 0 else nc.scalar
                            eng.dma_start(
                                x_tile[:], xT_t[:, :, lo:lo + SQ])
                        xq.append(x_tile)
                    for ot in range(n_qk):
                        if ot not in w_tiles:
                            w_tiles[ot] = wpool.tile([P, n_ht, P], MD,
                                                     tag="w", name="w_tile")
                            nc.gpsimd.dma_start(
                                w_tiles[ot][:],
                                wqkT_t[:, :, ot * P:(ot + 1) * P])
                        w_tile = w_tiles[ot]
                        if half == 0 and ot == 1:
                            nc.gpsimd.dma_start(wv_sb[:], wvT_t[:])
                        for sb in range(sb_per_half):
                            ps = pspool.tile([P, SQ], F32, tag="qkv")
                            for t in range(n_ht):
                                nc.tensor.matmul(
                                    ps[:],
                                    w_tile[:, t, :],
                                    xq[sb][:, t, :],
                                    start=(t == 0), stop=(t == n_ht - 1))
                            # fold the softmax scale into q at eviction;
                            # write straight into the resident qk tile
                            mul = scale if ot < dpc // P else 1.0
                            lo = half * s_half + sb * SQ
                            nc.scalar.mul(qk_all[:, ot, lo:lo + SQ],
                                          ps[:], mul)
                    # v: psum [s=128, dpc] accumulated over h-tiles
                    for sti in range(s_half // P):
                        st_g = half * (s_half // P) + sti
                        sb, off = (sti * P) // SQ, (sti * P) % SQ
                        ps_v = pspool.tile([P, dpc], F32, tag="qkv")
                        for t in range(n_ht):
                            nc.tensor.matmul(
                                ps_v[:],
                                xq[sb][:, t, off:off + P],
                                wv_sb[:, t, :],
                                start=(t == 0), stop=(t == n_ht - 1))
                        nc.vector.tensor_copy(v_sb[:, st_g, :], ps_v[:])

            # ====== phases 2-4: attention / chunked AllGather / o_proj ======
            maskT_tp = maskT.ap().rearrange("(t p) q -> p t q", p=P)
            mask_pairs = sorted({(t - t % 2, b)
                                 for (t, b), v in block_cls.items()
                                 if v == "mask"}, key=lambda k: (k[1], k[0]))
            with (
                tc.tile_pool(name="at_mask", bufs=1) as mpool,
                tc.tile_pool(name="at_exp", bufs=4) as epool,
                tc.tile_pool(name="at_out", bufs=3) as opool,
                tc.tile_pool(name="at_r", bufs=2) as rpool,
                tc.tile_pool(name="op_w", bufs=1) as owpool,
                tc.tile_pool(name="op_ct", bufs=48) as ctpool,
                tc.tile_pool(name="op_part", bufs=5) as partpool,
                tc.tile_pool(name="op_stage", bufs=3) as ospool,
                tc.tile_pool(name="at_ps_s", bufs=2, space="PSUM") as aps_s,
                tc.tile_pool(name="at_ps_o", bufs=1, space="PSUM") as aps_o,
                tc.tile_pool(name="at_ps_r", bufs=1, space="PSUM") as aps_r,
                tc.tile_pool(name="op_ps", bufs=2, space="PSUM") as opspool,
            ):
                wo_sb = owpool.tile([P, n_ht, dpc], MD, tag="wo")
                # causal-mask pair blocks resident in SBUF; copied into the
                # PSUM banks ahead of each masked scores pair (start=False)
                mp_tiles = {}
                if mask_pairs:
                    mt_all = mpool.tile([P, len(mask_pairs), 2, SQ], F32,
                                        tag="mask")
                    for i, (t0, b) in enumerate(mask_pairs):
                        nc.gpsimd.dma_start(
                            mt_all[:, i, :, :],
                            maskT_tp[:, t0:t0 + 2, b * SQ:(b + 1) * SQ])
                        mp_tiles[(t0, b)] = mt_all[:, i, :, :]

                ct_tiles = {}   # (b, pp) -> list of 16 SBUF k-tile views
                partials = {}   # st -> SBUF partial o_proj accumulator

                def prefetch_ct(b, pp):
                    ct_t = ct_b[b][pp].ap().rearrange(
                        "(t p) s -> p t s", p=P)
                    tiles = []
                    for t in range(n_hh):
                        c_t = ctpool.tile([P, SQ], MD, tag="ct")
                        nc.sync.dma_start(c_t[:], ct_t[:, t, :])
                        tiles.append(c_t)
                    ct_tiles[(b, pp)] = tiles

                def emit_oproj_half(bprev, st, pp):
                    cts = ct_tiles[(bprev, pp)]
                    ps = opspool.tile([P, dpc], F32, tag="op", name="op_ps")
                    for tt in range(n_hh):
                        nc.tensor.matmul(
                            ps[:],
                            cts[tt][:, st * P:(st + 1) * P],
                            wo_sb[:, pp * n_hh + tt, :],
                            start=(tt == 0), stop=(tt == n_hh - 1))
                    if pp == 0:
                        part = partpool.tile([P, dpc], F32, tag="part",
                                             name="part")
                        nc.vector.tensor_copy(part[:], ps[:])
                        partials[st] = part
                    else:
                        ob = ospool.tile([P, dpc], F32, tag="ostage",
                                         name="ostage")
                        nc.vector.tensor_add(ob[:], partials[st][:], ps[:])
                        nc.scalar.dma_start(
                            out_cols.ap()[bprev * SQ + st * P:
                                          bprev * SQ + (st + 1) * P, :],
                            ob[:])

                def head_attention(h, b, ts_here):
                    q_sl = qk_all[:, h, b * SQ:(b + 1) * SQ]
                    ps_o = aps_o.tile([P, SQ], F32, tag="out", name="ps_o")
                    ps_row = aps_r.tile([P, SQ], F32, tag="row",
                                        name="ps_row")
                    npair = len(ts_here) // 2
                    last_pair = npair - 1

                    def emit_evrow(pi, ex, t0, t1):
                        for j, t in ((0, t0), (1, t1)):
                            first = pi == 0 and j == 0
                            last = pi == last_pair and j == 1
                            nc.tensor.matmul(
                                ps_o[:], v_sb[:, t, h * P:(h + 1) * P],
                                ex[:, j, :], start=first, stop=last)
                            nc.tensor.matmul(
                                ps_row[:], ones_sq[:], ex[:, j, :],
                                start=first, stop=last)

                    pend = None
                    for pi in range(npair):
                        t0, t1 = ts_here[2 * pi], ts_here[2 * pi + 1]
                        ps_s = aps_s.tile([P, 2, SQ], F32, tag="scores",
                                          name="ps_s")
                        masked = block_cls[(t0, b)] == "mask"
                        assert masked == (block_cls[(t1, b)] == "mask")
                        if masked:
                            # preload the additive mask into both PSUM
                            # banks; the scores matmuls accumulate onto it
                            nc.vector.tensor_copy(
                                ps_s[:], mp_tiles[(t0, b)])
                        for j, t in ((0, t0), (1, t1)):
                            nc.tensor.matmul(
                                ps_s[:, j, :],
                                qk_all[:, hpc + h, t * P:(t + 1) * P],
                                q_sl, start=not masked, stop=True,
                                skip_group_check=masked)
                        ex = epool.tile([P, 2, SQ], MD, tag="exp",
                                        name="ex")
                        nc.scalar.activation(
                            ex[:], ps_s[:],
                            mybir.ActivationFunctionType.Exp)
                        if pend is not None:
                            emit_evrow(*pend)
                        pend = (pi, ex, t0, t1)
                    emit_evrow(*pend)

                    recip = rpool.tile([P, SQ], F32, tag="recip",
                                       name="recip")
                    nc.vector.reciprocal_approx_fast(recip[:], ps_row[:])
                    ob = opool.tile([P, SQ], MD, tag="ob", name="ob")
                    nc.vector.tensor_mul(ob[:], ps_o[:], recip[:])
                    nc.scalar.dma_start(
                        gat_b[b][h // 2].ap()[(h % 2) * P:
                                              (h % 2 + 1) * P, :], ob[:])

                for b in range(n_sq):
                    ts_here = [t for t in range(n_st) if (t, b) in block_cls]
                    assert ts_here and len(ts_here) % 2 == 0
                    for h in range(hpc):
                        head_attention(h, b, ts_here)
                        if h % 2 == 1:
                            pp = h // 2
                            nc.gpsimd.collective_compute(
                                "AllGather", mybir.AluOpType.bypass,
                                replica_groups=[list(range(N_CORES))],
                                ins=[gat_b[b][pp].ap().opt()],
                                outs=[ct_b[b][pp].ap().opt()])
                            prefetch_ct(b, pp)
                        if b == 0 and h == 0:
                            # wo arrives while block 0's attention runs,
                            # off the startup critical path
                            nc.gpsimd.dma_start(wo_sb[:], woT_t[:])
                        if b > 0:
                            # o_proj of the previous block: two
                            # half-contraction chains per head; gather-half
                            # pp=1 only after its collective had a full
                            # half-block of attention to land
                            for ch in (2 * h, 2 * h + 1):
                                st, pp = ch % 4, ch // 4
                                emit_oproj_half(b - 1, st, pp)
                    del ts_here
                # tail: o_proj of the final block (pp0 gathered mid-block,
                # pp1 right above)
                bl = n_sq - 1
                for ch in range(8):
                    st, pp = ch % 4, ch // 4
                    emit_oproj_half(bl, st, pp)

    nc.compile()
    return nc


def _classify_blocks(maskT_np, S):
    """Classify each [128, SQ] scoresT block of the (transposed) mask."""
    cls = {}
    for t in range(S // P):
        rows = maskT_np[t * P:(t + 1) * P]
        for b in range(S // SQ):
            blk = rows[:, b * SQ:(b + 1) * SQ]
            if np.all(blk <= -1e30):
                continue                      # fully masked: skip compute
            if np.all(blk == 0.0):
                cls[(t, b)] = "plain"
            else:
                cls[(t, b)] = "mask"
    return cls


def make_in_maps(hidden_states, attention_mask, w_pack, w_o):
    B, S, H = hidden_states.shape
    hpc = NUM_HEADS // N_CORES
    dpc = hpc * HEAD_DIM
    np_md = mybir.dt.np(_mm_dtype(MM_MODE))
    xT = np.ascontiguousarray(hidden_states[0].T).astype(np_md)
    maskT_np = np.ascontiguousarray(
        np.broadcast_to(attention_mask, (1, 1, S, S))[0, 0].T,
        dtype=np.float32)
    # w_o rows permuted to match the head-pair AllGather layout:
    # [pp][rank][head-in-pair] blocks of 128
    perm = np.concatenate(
        [np.arange(128 * (4 * r + 2 * pp + hh),
                   128 * (4 * r + 2 * pp + hh) + 128)
         for pp in (0, 1) for r in range(N_CORES) for hh in (0, 1)])
    in_maps = []
    for c in range(N_CORES):
        sl = slice(c * dpc, (c + 1) * dpc)
        wqk_c = np.concatenate(
            [w_pack[0 * H:1 * H][sl], w_pack[1 * H:2 * H][sl]], axis=0)
        woT_c = np.ascontiguousarray(w_o[sl].T)[perm]
        in_maps.append({
            "xT": xT,
            "wqkT": np.ascontiguousarray(wqk_c.T).astype(np_md),
            "wvT": np.ascontiguousarray(w_pack[2 * H:3 * H][sl].T
                                        ).astype(np_md),
            "maskT": maskT_np,
            "woT": np.ascontiguousarray(woT_c).astype(np_md),
        })
    return in_maps, maskT_np


def kernel(hidden_states, attention_mask, w_pack, w_o):
    B, S, H = hidden_states.shape
    assert B == 1 and H == NUM_HEADS * HEAD_DIM
    assert S % (2 * SQ) == 0

    in_maps, maskT_np = make_in_maps(hidden_states, attention_mask,
                                     w_pack, w_o)
    block_cls = _classify_blocks(maskT_np, S)

    key = (S, H, tuple(sorted(block_cls.items())), MM_MODE)
    if key not in _CACHE:
        _CACHE[key] = build(S, H, block_cls, MM_MODE)
    nc = _CACHE[key]

    res = run_bass_kernel_spmd(nc, in_maps, core_ids=list(range(N_CORES)))
    out = np.concatenate(
        [res.results[c]["out_cols"] for c in range(N_CORES)], axis=1)
    return out.reshape(1, S, H).astype(np.float32)


# revision 13
# speedup vs baseline: 1.0394x; 1.0274x over previous
"""Baichuan attention on 8 Trainium2 NeuronCores — tensor-parallel over heads.

Sharding: core c computes heads [4c, 4c+4): its slice of the fused QKV
projection, attention for those heads, then 1/8 of o_proj's output columns
after an AllGather of the per-core context slices (moves 4MB/rank instead of
a 32MB AllReduce of partial sums; mathematically identical to the module's
world_size logic).

Layout: scores are computed transposed (scoresT[k, q] blocks) so the PE
contraction dim always sits on SBUF partitions and every matmul streams a
512-wide moving operand. Matmul operands are fp16 (1 cyc/row on the PE) with
fp32 PSUM accumulation.

v2 pipeline notes (from the v1 trace: PE 87% busy, 102us of gaps):
- scores land in [128, 2x512] two-bank PSUM tiles so ONE exp activation
  covers a k-tile pair: halves the Scalar engine's per-tile overhead, which
  otherwise paces the PE in the attention phase.
- the causal mask is DMA'd straight from DRAM into the PSUM banks and the
  scores matmul runs with start=False (accumulate onto the mask): the DVE
  mask-adds disappear.
- softmax denominators use reciprocal_approx_fast (1 DVE op, ~18 good bits)
  instead of reciprocal (4us each); the normalize multiply reads the PSUM
  accumulator directly. PSUM banks now recycle ~1us after the last matmul,
  which removes the 3-4us PE stalls v1 had on every head.
- o_proj is emitted as 8 half-contraction chains per block, two after each
  head's attention, split by AllGather half so a chain only ever consumes a
  collective that finished a full block earlier. PSUM: scores 2x2 banks,
  ps_o 2, ps_row 1, o_proj 1 = 8.
- DMAs are spread across engine queues (x: sync+scalar, weights: vector,
  gathers/out: scalar, mask preloads: vector) and the first weight/x tiles
  are split so the PE starts ~1.5us into the kernel instead of 16us.
"""

import numpy as np

import concourse.bacc as bacc
import concourse.mybir as mybir
import concourse.tile as tile
from concourse.bass_utils import run_bass_kernel_spmd

F32 = mybir.dt.float32

N_CORES = 8
NUM_HEADS = 32
HEAD_DIM = 128
P = 128          # SBUF partitions / PE contraction tile
SQ = 512         # s_q block width (PSUM bank = 512 fp32)
MM_MODE = "f16"  # 'f16' | 'f32' (operand dtype for matmuls)

_CACHE: dict = {}


def _mm_dtype(mode):
    return {"f16": mybir.dt.float16, "f32": F32}[mode]


def build(S, H, block_cls, mode=MM_MODE):
    """Build the SPMD program. block_cls[(t, b)] = 'plain' | 'mask' for every
    computed scoresT block ([128 s_k] x [SQ s_q]); absent = fully masked, skip.
    """
    MD = _mm_dtype(mode)
    hpc = NUM_HEADS // N_CORES          # heads per core
    dpc = hpc * HEAD_DIM                # per-core slice of the hidden dim
    n_ht = H // P                       # contraction tiles for QKV/o_proj
    n_qk = 2 * dpc // P                 # q+k output tiles
    n_sq = S // SQ                      # s_q blocks
    n_st = S // P                       # s_k tiles
    scale = 1.0 / np.sqrt(np.float32(HEAD_DIM))
    s_half = S // 2
    sb_per_half = s_half // SQ
    n_hh = n_ht // 2                    # o_proj k-tiles per gather half

    nc = bacc.Bacc("TRN2", target_bir_lowering=False, debug=False,
                   num_devices=N_CORES)

    xT = nc.dram_tensor("xT", [H, S], MD, kind="ExternalInput")
    wqkT = nc.dram_tensor("wqkT", [H, 2 * dpc], MD, kind="ExternalInput")
    wvT = nc.dram_tensor("wvT", [H, dpc], MD, kind="ExternalInput")
    maskT = nc.dram_tensor("maskT", [S, S], F32, kind="ExternalInput")
    woT = nc.dram_tensor("woT", [H, dpc], MD, kind="ExternalInput")
    out_cols = nc.dram_tensor("out_cols", [S, dpc], F32, kind="ExternalOutput")

    # AllGather in head-pair chunks: gat[b][pp] holds local heads
    # {2pp, 2pp+1} for s_q block b; ct[b][pp] gathers those pairs from all
    # ranks. o_proj consumes them against host-permuted w_o rows.
    gat_b = [[nc.dram_tensor(f"gat_{b}_{pp}", [dpc // 2, SQ], MD)
              for pp in range(2)] for b in range(n_sq)]
    ct_b = [[nc.dram_tensor(f"ct_{b}_{pp}", [H // 2, SQ], MD,
                            addr_space="Shared") for pp in range(2)]
            for b in range(n_sq)]
    # tiny warmup collective: absorbs the one-time CC-stream/HAM setup
    # (~14us) during phase 1 so the first real AllGather runs at steady
    # latency
    warm_in = nc.dram_tensor("warm_in", [1, 64], MD)
    warm_out = nc.dram_tensor("warm_out", [1, 64 * N_CORES], MD,
                              addr_space="Shared")

    xT_t = xT.ap().rearrange("(t p) s -> p t s", p=P)
    wqkT_t = wqkT.ap().rearrange("(t p) o -> p t o", p=P)
    wvT_t = wvT.ap().rearrange("(t p) o -> p t o", p=P)
    woT_t = woT.ap().rearrange("(t p) j -> p t j", p=P)

    with tile.TileContext(nc) as tc:
        with (
            tc.tile_pool(name="consts", bufs=1) as cpool,
            tc.tile_pool(name="span", bufs=1) as span,
        ):
            ones_f = cpool.tile([P, P], F32, tag="ones_f")
            nc.gpsimd.memset(ones_f[:], 1.0)
            ones_sq = cpool.tile([P, P], MD, tag="ones_sq")
            nc.scalar.copy(ones_sq[:], ones_f[:])
            nc.gpsimd.collective_compute(
                "AllGather", mybir.AluOpType.bypass,
                replica_groups=[list(range(N_CORES))],
                ins=[warm_in.ap().opt()], outs=[warm_out.ap().opt()])

            # v ([s_k, d] natural, all heads) and q/k (transposed, all heads)
            # live in SBUF across phases 1-2; QKV evictions write them
            # directly (no DRAM bounce)
            v_sb = span.tile([P, n_st, dpc], MD, tag="v")
            qk_all = span.tile([P, n_qk, S], MD, tag="qk")

            # =============== phase 1: QKV projection ===============
            # q/k in transposed orientation -> resident qk_all; v in natural
            # orientation (x stationary, Wv moving) -> resident v_sb.
            with (
                tc.tile_pool(name="qkv_x", bufs=1) as xpool,
                tc.tile_pool(name="qkv_w", bufs=3) as wpool,
                tc.tile_pool(name="qkv_wv", bufs=1) as wvpool,
                tc.tile_pool(name="qkv_ps", bufs=4, space="PSUM") as pspool,
            ):
                wv_sb = wvpool.tile([P, n_ht, dpc], MD, tag="wv")
                for half in range(2):
                    # DMA queues: sync feeds w-tile 0 + x quarter 0 (fine-
                    # grained, interleaved so the PE starts ~1.5us in),
                    # scalar feeds x quarter 1 in parallel, gpsimd carries
                    # the remaining weight tiles + wv off the startup path.
                    w_tiles = {}
                    w_tiles[0] = wpool.tile([P, n_ht, P], MD, tag="w",
                                            name="w_tile")
                    xq = []
                    for sb in range(sb_per_half):
                        x_tile = xpool.tile([P, n_ht, SQ], MD, tag="x",
                                            bufs=3, name="x_tile")
                        lo = half * s_half + sb * SQ
                        if half == 0 and sb == 0:
                            for c in range(4):
                                nc.sync.dma_start(
                                    w_tiles[0][:, 8 * c:8 * (c + 1), :],
                                    wqkT_t[:, 8 * c:8 * (c + 1), 0:P])
                                nc.sync.dma_start(
                                    x_tile[:, 2 * c:2 * (c + 1), :],
                                    xT_t[:, 2 * c:2 * (c + 1), lo:lo + SQ])
                            nc.sync.dma_start(
                                x_tile[:, 8:, :], xT_t[:, 8:, lo:lo + SQ])
                        elif sb == 0:
                            nc.gpsimd.dma_start(
                                w_tiles[0][:], wqkT_t[:, :, 0:P])
                            nc.sync.dma_start(
                                x_tile[:], xT_t[:, :, lo:lo + SQ])
                        else:
                            nc.scalar.dma_start(
                                x_tile[:], xT_t[:, :, lo:lo + SQ])
                        xq.append(x_tile)
                    for ot in range(n_qk):
                        if ot not in w_tiles:
                            w_tiles[ot] = wpool.tile([P, n_ht, P], MD,
                                                     tag="w", name="w_tile")
                            nc.gpsimd.dma_start(
                                w_tiles[ot][:],
                                wqkT_t[:, :, ot * P:(ot + 1) * P])
                        w_tile = w_tiles[ot]
                        if half == 0 and ot == 5:
                            nc.gpsimd.dma_start(wv_sb[:], wvT_t[:])
                        for sb in range(sb_per_half):
                            ps = pspool.tile([P, SQ], F32, tag="qkv")
                            for t in range(n_ht):
                                nc.tensor.matmul(
                                    ps[:],
                                    w_tile[:, t, :],
                                    xq[sb][:, t, :],
                                    start=(t == 0), stop=(t == n_ht - 1))
                            # fold the softmax scale into q at eviction;
                            # write straight into the resident qk tile
                            mul = scale if ot < dpc // P else 1.0
                            lo = half * s_half + sb * SQ
                            nc.scalar.mul(qk_all[:, ot, lo:lo + SQ],
                                          ps[:], mul)
                    # v: psum [s=128, dpc] accumulated over h-tiles
                    for sti in range(s_half // P):
                        st_g = half * (s_half // P) + sti
                        sb, off = (sti * P) // SQ, (sti * P) % SQ
                        ps_v = pspool.tile([P, dpc], F32, tag="qkv")
                        for t in range(n_ht):
                            nc.tensor.matmul(
                                ps_v[:],
                                xq[sb][:, t, off:off + P],
                                wv_sb[:, t, :],
                                start=(t == 0), stop=(t == n_ht - 1))
                        nc.vector.tensor_copy(v_sb[:, st_g, :], ps_v[:])

            # ====== phases 2-4: attention / chunked AllGather / o_proj ======
            maskT_tp = maskT.ap().rearrange("(t p) q -> p t q", p=P)
            mask_pairs = sorted({(t - t % 2, b)
                                 for (t, b), v in block_cls.items()
                                 if v == "mask"}, key=lambda k: (k[1], k[0]))
            with (
                tc.tile_pool(name="at_mask", bufs=1) as mpool,
                tc.tile_pool(name="at_exp", bufs=4) as epool,
                tc.tile_pool(name="at_out", bufs=3) as opool,
                tc.tile_pool(name="at_r", bufs=2) as rpool,
                tc.tile_pool(name="op_w", bufs=1) as owpool,
                tc.tile_pool(name="op_ct", bufs=48) as ctpool,
                tc.tile_pool(name="op_part", bufs=5) as partpool,
                tc.tile_pool(name="op_stage", bufs=3) as ospool,
                tc.tile_pool(name="at_ps_s", bufs=2, space="PSUM") as aps_s,
                tc.tile_pool(name="at_ps_o", bufs=1, space="PSUM") as aps_o,
                tc.tile_pool(name="at_ps_r", bufs=1, space="PSUM") as aps_r,
                tc.tile_pool(name="op_ps", bufs=2, space="PSUM") as opspool,
            ):
                wo_sb = owpool.tile([P, n_ht, dpc], MD, tag="wo")
                # causal-mask pair blocks resident in SBUF; copied into the
                # PSUM banks ahead of each masked scores pair (start=False)
                mp_tiles = {}
                if mask_pairs:
                    mt_all = mpool.tile([P, len(mask_pairs), 2, SQ], F32,
                                        tag="mask")
                    for i, (t0, b) in enumerate(mask_pairs):
                        nc.gpsimd.dma_start(
                            mt_all[:, i, :, :],
                            maskT_tp[:, t0:t0 + 2, b * SQ:(b + 1) * SQ])
                        mp_tiles[(t0, b)] = mt_all[:, i, :, :]

                ct_tiles = {}   # (b, pp) -> list of 16 SBUF k-tile views
                partials = {}   # st -> SBUF partial o_proj accumulator

                def prefetch_ct(b, pp):
                    ct_t = ct_b[b][pp].ap().rearrange(
                        "(t p) s -> p t s", p=P)
                    tiles = []
                    for t in range(n_hh):
                        c_t = ctpool.tile([P, SQ], MD, tag="ct")
                        nc.sync.dma_start(c_t[:], ct_t[:, t, :])
                        tiles.append(c_t)
                    ct_tiles[(b, pp)] = tiles

                def emit_oproj_half(bprev, st, pp):
                    cts = ct_tiles[(bprev, pp)]
                    ps = opspool.tile([P, dpc], F32, tag="op", name="op_ps")
                    for tt in range(n_hh):
                        nc.tensor.matmul(
                            ps[:],
                            cts[tt][:, st * P:(st + 1) * P],
                            wo_sb[:, pp * n_hh + tt, :],
                            start=(tt == 0), stop=(tt == n_hh - 1))
                    if pp == 0:
                        part = partpool.tile([P, dpc], F32, tag="part",
                                             name="part")
                        nc.vector.tensor_copy(part[:], ps[:])
                        partials[st] = part
                    else:
                        ob = ospool.tile([P, dpc], F32, tag="ostage",
                                         name="ostage")
                        nc.vector.tensor_add(ob[:], partials[st][:], ps[:])
                        nc.scalar.dma_start(
                            out_cols.ap()[bprev * SQ + st * P:
                                          bprev * SQ + (st + 1) * P, :],
                            ob[:])

                def head_attention(h, b, ts_here):
                    q_sl = qk_all[:, h, b * SQ:(b + 1) * SQ]
                    ps_o = aps_o.tile([P, SQ], F32, tag="out", name="ps_o")
                    ps_row = aps_r.tile([P, SQ], F32, tag="row",
                                        name="ps_row")
                    npair = len(ts_here) // 2
                    last_pair = npair - 1

                    def emit_evrow(pi, ex, t0, t1):
                        for j, t in ((0, t0), (1, t1)):
                            first = pi == 0 and j == 0
                            last = pi == last_pair and j == 1
                            nc.tensor.matmul(
                                ps_o[:], v_sb[:, t, h * P:(h + 1) * P],
                                ex[:, j, :], start=first, stop=last)
                            nc.tensor.matmul(
                                ps_row[:], ones_sq[:], ex[:, j, :],
                                start=first, stop=last)

                    pend = None
                    for pi in range(npair):
                        t0, t1 = ts_here[2 * pi], ts_here[2 * pi + 1]
                        ps_s = aps_s.tile([P, 2, SQ], F32, tag="scores",
                                          name="ps_s")
                        masked = block_cls[(t0, b)] == "mask"
                        assert masked == (block_cls[(t1, b)] == "mask")
                        if masked:
                            # preload the additive mask into both PSUM
                            # banks; the scores matmuls accumulate onto it
                            nc.vector.tensor_copy(
                                ps_s[:], mp_tiles[(t0, b)])
                        for j, t in ((0, t0), (1, t1)):
                            nc.tensor.matmul(
                                ps_s[:, j, :],
                                qk_all[:, hpc + h, t * P:(t + 1) * P],
                                q_sl, start=not masked, stop=True,
                                skip_group_check=masked)
                        ex = epool.tile([P, 2, SQ], MD, tag="exp",
                                        name="ex")
                        nc.scalar.activation(
                            ex[:], ps_s[:],
                            mybir.ActivationFunctionType.Exp)
                        if pend is not None:
                            emit_evrow(*pend)
                        pend = (pi, ex, t0, t1)
                    emit_evrow(*pend)

                    recip = rpool.tile([P, SQ], F32, tag="recip",
                                       name="recip")
                    nc.vector.reciprocal_approx_fast(recip[:], ps_row[:])
                    ob = opool.tile([P, SQ], MD, tag="ob", name="ob")
                    nc.vector.tensor_mul(ob[:], ps_o[:], recip[:])
                    nc.scalar.dma_start(
                        gat_b[b][h // 2].ap()[(h % 2) * P:
                                              (h % 2 + 1) * P, :], ob[:])

                for b in range(n_sq):
                    ts_here = [t for t in range(n_st) if (t, b) in block_cls]
                    assert ts_here and len(ts_here) % 2 == 0
                    for h in range(hpc):
                        head_attention(h, b, ts_here)
                        if h % 2 == 1:
                            pp = h // 2
                            nc.gpsimd.collective_compute(
                                "AllGather", mybir.AluOpType.bypass,
                                replica_groups=[list(range(N_CORES))],
                                ins=[gat_b[b][pp].ap().opt()],
                                outs=[ct_b[b][pp].ap().opt()])
                            prefetch_ct(b, pp)
                        if b == 0 and h == 0:
                            # wo arrives while block 0's attention runs,
                            # off the startup critical path
                            nc.gpsimd.dma_start(wo_sb[:], woT_t[:])
                        if b > 0 and h > 0:
                            # o_proj of the previous block: two
                            # half-contraction chains after heads 1-3 (the
                            # last two run post-block), so a chain only
                            # starts once its AllGather half has had time
                            # to land
                            for ch in (2 * h - 2, 2 * h - 1):
                                st, pp = ch % 4, ch // 4
                                emit_oproj_half(b - 1, st, pp)
                    if b > 0:
                        for ch in (6, 7):
                            st, pp = ch % 4, ch // 4
                            emit_oproj_half(b - 1, st, pp)
                # tail: o_proj of the final block (pp0 gathered mid-block,
                # pp1 right above)
                bl = n_sq - 1
                for ch in range(8):
                    st, pp = ch % 4, ch // 4
                    emit_oproj_half(bl, st, pp)

    nc.compile()
    return nc


def _classify_blocks(maskT_np, S):
    """Classify each [128, SQ] scoresT block of the (transposed) mask."""
    cls = {}
    for t in range(S // P):
        rows = maskT_np[t * P:(t + 1) * P]
        for b in range(S // SQ):
            blk = rows[:, b * SQ:(b + 1) * SQ]
            if np.all(blk <= -1e30):
                continue                      # fully masked: skip compute
            if np.all(blk == 0.0):
                cls[(t, b)] = "plain"
            else:
                cls[(t, b)] = "mask"
    return cls


def make_in_maps(hidden_states, attention_mask, w_pack, w_o):
    B, S, H = hidden_states.shape
    hpc = NUM_HEADS // N_CORES
    dpc = hpc * HEAD_DIM
    np_md = mybir.dt.np(_mm_dtype(MM_MODE))
    xT = np.ascontiguousarray(hidden_states[0].T).astype(np_md)
    maskT_np = np.ascontiguousarray(
        np.broadcast_to(attention_mask, (1, 1, S, S))[0, 0].T,
        dtype=np.float32)
    # w_o rows permuted to match the head-pair AllGather layout:
    # [pp][rank][head-in-pair] blocks of 128
    perm = np.concatenate(
        [np.arange(128 * (4 * r + 2 * pp + hh),
                   128 * (4 * r + 2 * pp + hh) + 128)
         for pp in (0, 1) for r in range(N_CORES) for hh in (0, 1)])
    in_maps = []
    for c in range(N_CORES):
        sl = slice(c * dpc, (c + 1) * dpc)
        wqk_c = np.concatenate(
            [w_pack[0 * H:1 * H][sl], w_pack[1 * H:2 * H][sl]], axis=0)
        woT_c = np.ascontiguousarray(w_o[sl].T)[perm]
        in_maps.append({
            "xT": xT,
            "wqkT": np.ascontiguousarray(wqk_c.T).astype(np_md),
            "wvT": np.ascontiguousarray(w_pack[2 * H:3 * H][sl].T
                                        ).astype(np_md),
            "maskT": maskT_np,
            "woT": np.ascontiguousarray(woT_c).astype(np_md),
        })
    return in_maps, maskT_np


def kernel(hidden_states, attention_mask, w_pack, w_o):
    B, S, H = hidden_states.shape
    assert B == 1 and H == NUM_HEADS * HEAD_DIM
    assert S % (2 * SQ) == 0

    in_maps, maskT_np = make_in_maps(hidden_states, attention_mask,
                                     w_pack, w_o)
    block_cls = _classify_blocks(maskT_np, S)

    key = (S, H, tuple(sorted(block_cls.items())), MM_MODE)
    if key not in _CACHE:
        _CACHE[key] = build(S, H, block_cls, MM_MODE)
    nc = _CACHE[key]

    res = run_bass_kernel_spmd(nc, in_maps, core_ids=list(range(N_CORES)))
    out = np.concatenate(
        [res.results[c]["out_cols"] for c in range(N_CORES)], axis=1)
    return out.reshape(1, S, H).astype(np.float32)


# revision 17
# speedup vs baseline: 1.0669x; 1.0265x over previous
"""Baichuan attention on 8 Trainium2 NeuronCores — tensor-parallel over heads.

Sharding: core c computes heads [4c, 4c+4): its slice of the fused QKV
projection, attention for those heads, then 1/8 of o_proj's output columns
after an AllGather of the per-core context slices (moves 4MB/rank instead of
a 32MB AllReduce of partial sums; mathematically identical to the module's
world_size logic).

Layout: scores are computed transposed (scoresT[k, q] blocks) so the PE
contraction dim always sits on SBUF partitions and every matmul streams a
512-wide moving operand. Matmul operands are fp16 (1 cyc/row on the PE) with
fp32 PSUM accumulation.

v2 pipeline notes (from the v1 trace: PE 87% busy, 102us of gaps):
- scores land in [128, 2x512] two-bank PSUM tiles so ONE exp activation
  covers a k-tile pair: halves the Scalar engine's per-tile overhead, which
  otherwise paces the PE in the attention phase.
- the causal mask is DMA'd straight from DRAM into the PSUM banks and the
  scores matmul runs with start=False (accumulate onto the mask): the DVE
  mask-adds disappear.
- softmax denominators use reciprocal_approx_fast (1 DVE op, ~18 good bits)
  instead of reciprocal (4us each); the normalize multiply reads the PSUM
  accumulator directly. PSUM banks now recycle ~1us after the last matmul,
  which removes the 3-4us PE stalls v1 had on every head.
- o_proj is emitted as 8 half-contraction chains per block, two after each
  head's attention, split by AllGather half so a chain only ever consumes a
  collective that finished a full block earlier. PSUM: scores 2x2 banks,
  ps_o 2, ps_row 1, o_proj 1 = 8.
- DMAs are spread across engine queues (x: sync+scalar, weights: vector,
  gathers/out: scalar, mask preloads: vector) and the first weight/x tiles
  are split so the PE starts ~1.5us into the kernel instead of 16us.
"""

import numpy as np

import concourse.bacc as bacc
import concourse.mybir as mybir
import concourse.tile as tile
from concourse.bass_utils import run_bass_kernel_spmd

F32 = mybir.dt.float32

N_CORES = 8
NUM_HEADS = 32
HEAD_DIM = 128
P = 128          # SBUF partitions / PE contraction tile
SQ = 512         # s_q block width (PSUM bank = 512 fp32)
MM_MODE = "f16"  # 'f16' | 'f32' (operand dtype for matmuls)

_CACHE: dict = {}


def _mm_dtype(mode):
    return {"f16": mybir.dt.float16, "f32": F32}[mode]


def build(S, H, block_cls, mode=MM_MODE):
    """Build the SPMD program. block_cls[(t, b)] = 'plain' | 'mask' for every
    computed scoresT block ([128 s_k] x [SQ s_q]); absent = fully masked, skip.
    """
    MD = _mm_dtype(mode)
    hpc = NUM_HEADS // N_CORES          # heads per core
    dpc = hpc * HEAD_DIM                # per-core slice of the hidden dim
    n_ht = H // P                       # contraction tiles for QKV/o_proj
    n_qk = 2 * dpc // P                 # q+k output tiles
    n_sq = S // SQ                      # s_q blocks
    n_st = S // P                       # s_k tiles
    scale = 1.0 / np.sqrt(np.float32(HEAD_DIM))
    s_half = S // 2
    sb_per_half = s_half // SQ
    n_hh = n_ht // 2                    # o_proj k-tiles per gather half

    nc = bacc.Bacc("TRN2", target_bir_lowering=False, debug=False,
                   num_devices=N_CORES)

    xT = nc.dram_tensor("xT", [H, S], MD, kind="ExternalInput")
    wqkT = nc.dram_tensor("wqkT", [H, 2 * dpc], MD, kind="ExternalInput")
    wvT = nc.dram_tensor("wvT", [H, dpc], MD, kind="ExternalInput")
    maskT = nc.dram_tensor("maskT", [S, S], MD, kind="ExternalInput")
    woT = nc.dram_tensor("woT", [H, dpc], MD, kind="ExternalInput")
    out_cols = nc.dram_tensor("out_cols", [S, dpc], F32, kind="ExternalOutput")

    # AllGather in head-pair chunks: gat[b][pp] holds local heads
    # {2pp, 2pp+1} for s_q block b; ct[b][pp] gathers those pairs from all
    # ranks. o_proj consumes them against host-permuted w_o rows.
    gat_b = [[nc.dram_tensor(f"gat_{b}_{pp}", [dpc // 2, SQ], MD)
              for pp in range(2)] for b in range(n_sq)]
    ct_b = [[nc.dram_tensor(f"ct_{b}_{pp}", [H // 2, SQ], MD,
                            addr_space="Shared") for pp in range(2)]
            for b in range(n_sq)]
    # tiny warmup collective: absorbs the one-time CC-stream/HAM setup
    # (~14us) during phase 1 so the first real AllGather runs at steady
    # latency
    warm_in = nc.dram_tensor("warm_in", [1, 64], MD)
    warm_out = nc.dram_tensor("warm_out", [1, 64 * N_CORES], MD,
                              addr_space="Shared")

    xT_t = xT.ap().rearrange("(t p) s -> p t s", p=P)
    wqkT_t = wqkT.ap().rearrange("(t p) o -> p t o", p=P)
    wvT_t = wvT.ap().rearrange("(t p) o -> p t o", p=P)
    woT_t = woT.ap().rearrange("(t p) j -> p t j", p=P)

    with tile.TileContext(nc) as tc:
        with (
            tc.tile_pool(name="consts", bufs=1) as cpool,
            tc.tile_pool(name="span", bufs=1) as span,
        ):
            ones_f = cpool.tile([P, P], F32, tag="ones_f")
            nc.gpsimd.memset(ones_f[:], 1.0)
            ones_sq = cpool.tile([P, P], MD, tag="ones_sq")
            nc.scalar.copy(ones_sq[:], ones_f[:])
            nc.gpsimd.collective_compute(
                "AllGather", mybir.AluOpType.bypass,
                replica_groups=[list(range(N_CORES))],
                ins=[warm_in.ap().opt()], outs=[warm_out.ap().opt()])

            # v ([s_k, d] natural, all heads) and q/k (transposed, all heads)
            # live in SBUF across phases 1-2; QKV evictions write them
            # directly (no DRAM bounce)
            v_sb = span.tile([P, n_st, dpc], MD, tag="v")
            qk_all = span.tile([P, n_qk, S], MD, tag="qk")

            # =============== phase 1: QKV projection ===============
            # q/k in transposed orientation -> resident qk_all; v in natural
            # orientation (x stationary, Wv moving) -> resident v_sb.
            with (
                tc.tile_pool(name="qkv_x", bufs=1) as xpool,
                tc.tile_pool(name="qkv_w", bufs=3) as wpool,
                tc.tile_pool(name="qkv_wv", bufs=1) as wvpool,
                tc.tile_pool(name="qkv_ps", bufs=4, space="PSUM") as pspool,
            ):
                wv_sb = wvpool.tile([P, n_ht, dpc], MD, tag="wv")
                for half in range(2):
                    # sb-outer: all 8 output chains run against one resident
                    # x quarter, so the other quarter has a full 66us to
                    # arrive. Queues: sync feeds w-tile 0 + x quarter 0
                    # (fine-grained so the PE starts ~3us in), scalar feeds
                    # quarter 1, gpsimd streams the weight tiles (re-DMA'd
                    # per quarter: 8MB of extra reads buys a bufs=3 ring).
                    xq = []
                    for sb in range(sb_per_half):
                        x_tile = xpool.tile([P, n_ht, SQ], MD, tag="x",
                                            bufs=3, name="x_tile")
                        lo = half * s_half + sb * SQ
                        if half == 0 and sb == 0:
                            for t in range(8):
                                nc.sync.dma_start(
                                    x_tile[:, t, :],
                                    xT_t[:, t, lo:lo + SQ])
                            nc.sync.dma_start(
                                x_tile[:, 8:16, :],
                                xT_t[:, 8:16, lo:lo + SQ])
                            nc.scalar.dma_start(
                                x_tile[:, 16:, :], xT_t[:, 16:, lo:lo + SQ])
                        else:
                            eng = nc.sync if sb % 2 == 0 else nc.scalar
                            eng.dma_start(x_tile[:], xT_t[:, :, lo:lo + SQ])
                        xq.append(x_tile)
                    for sb in range(sb_per_half):
                        lo = half * s_half + sb * SQ
                        for ot in range(n_qk):
                            w_tile = wpool.tile([P, n_ht, P], MD, tag="w",
                                                name="w_tile")
                            if half == 0 and sb == 0 and ot == 0:
                                # ahead of the x stream on the sync queue
                                for c in range(4):
                                    nc.sync.dma_start(
                                        w_tile[:, 8 * c:8 * (c + 1), :],
                                        wqkT_t[:, 8 * c:8 * (c + 1), 0:P])
                            else:
                                nc.gpsimd.dma_start(
                                    w_tile[:],
                                    wqkT_t[:, :, ot * P:(ot + 1) * P])
                            ps = pspool.tile([P, SQ], F32, tag="qkv")
                            for t in range(n_ht):
                                nc.tensor.matmul(
                                    ps[:],
                                    w_tile[:, t, :],
                                    xq[sb][:, t, :],
                                    start=(t == 0), stop=(t == n_ht - 1))
                            # fold the softmax scale into q at eviction;
                            # write straight into the resident qk tile
                            mul = scale if ot < dpc // P else 1.0
                            nc.scalar.mul(qk_all[:, ot, lo:lo + SQ],
                                          ps[:], mul)
                        if half == 0 and sb == 0:
                            nc.gpsimd.dma_start(wv_sb[:], wvT_t[:])
                    # v: psum [s=128, dpc] accumulated over h-tiles
                    for sti in range(s_half // P):
                        st_g = half * (s_half // P) + sti
                        sb, off = (sti * P) // SQ, (sti * P) % SQ
                        ps_v = pspool.tile([P, dpc], F32, tag="qkv")
                        for t in range(n_ht):
                            nc.tensor.matmul(
                                ps_v[:],
                                xq[sb][:, t, off:off + P],
                                wv_sb[:, t, :],
                                start=(t == 0), stop=(t == n_ht - 1))
                        nc.vector.tensor_copy(v_sb[:, st_g, :], ps_v[:])

            # ====== phases 2-4: attention / chunked AllGather / o_proj ======
            maskT_tp = maskT.ap().rearrange("(t p) q -> p t q", p=P)
            mask_pairs = sorted({(t - t % 2, b)
                                 for (t, b), v in block_cls.items()
                                 if v == "mask"}, key=lambda k: (k[1], k[0]))
            with (
                tc.tile_pool(name="at_mask", bufs=1) as mpool,
                tc.tile_pool(name="at_exp", bufs=4) as epool,
                tc.tile_pool(name="at_out", bufs=3) as opool,
                tc.tile_pool(name="at_r", bufs=2) as rpool,
                tc.tile_pool(name="op_w", bufs=1) as owpool,
                tc.tile_pool(name="op_ct", bufs=36) as ctpool,
                tc.tile_pool(name="op_part", bufs=5) as partpool,
                tc.tile_pool(name="op_stage", bufs=3) as ospool,
                tc.tile_pool(name="at_ps_s", bufs=2, space="PSUM") as aps_s,
                tc.tile_pool(name="at_ps_o", bufs=1, space="PSUM") as aps_o,
                tc.tile_pool(name="at_ps_r", bufs=1, space="PSUM") as aps_r,
                tc.tile_pool(name="op_ps", bufs=2, space="PSUM") as opspool,
            ):
                wo_sb = owpool.tile([P, n_ht, dpc], MD, tag="wo")
                # causal-mask pair blocks resident in SBUF; copied into the
                # PSUM banks ahead of each masked scores pair (start=False)
                mp_tiles = {}
                if mask_pairs:
                    mt_all = mpool.tile([P, len(mask_pairs), 2, SQ], MD,
                                        tag="mask")
                    for i, (t0, b) in enumerate(mask_pairs):
                        nc.gpsimd.dma_start(
                            mt_all[:, i, :, :],
                            maskT_tp[:, t0:t0 + 2, b * SQ:(b + 1) * SQ])
                        mp_tiles[(t0, b)] = mt_all[:, i, :, :]

                ct_tiles = {}   # (b, pp) -> list of 16 SBUF k-tile views
                partials = {}   # st -> SBUF partial o_proj accumulator

                def prefetch_ct(b, pp):
                    ct_t = ct_b[b][pp].ap().rearrange(
                        "(t p) s -> p t s", p=P)
                    tiles = []
                    for t in range(n_hh):
                        c_t = ctpool.tile([P, SQ], MD, tag="ct")
                        nc.sync.dma_start(c_t[:], ct_t[:, t, :])
                        tiles.append(c_t)
                    ct_tiles[(b, pp)] = tiles

                def emit_oproj_half(bprev, st, pp):
                    cts = ct_tiles[(bprev, pp)]
                    ps = opspool.tile([P, dpc], F32, tag="op", name="op_ps")
                    for tt in range(n_hh):
                        nc.tensor.matmul(
                            ps[:],
                            cts[tt][:, st * P:(st + 1) * P],
                            wo_sb[:, pp * n_hh + tt, :],
                            start=(tt == 0), stop=(tt == n_hh - 1))
                    if pp == 0:
                        part = partpool.tile([P, dpc], F32, tag="part",
                                             name="part")
                        nc.vector.tensor_copy(part[:], ps[:])
                        partials[st] = part
                    else:
                        ob = ospool.tile([P, dpc], F32, tag="ostage",
                                         name="ostage")
                        nc.vector.tensor_add(ob[:], partials[st][:], ps[:])
                        nc.scalar.dma_start(
                            out_cols.ap()[bprev * SQ + st * P:
                                          bprev * SQ + (st + 1) * P, :],
                            ob[:])

                def head_attention(h, b, ts_here):
                    q_sl = qk_all[:, h, b * SQ:(b + 1) * SQ]
                    # ps_o/ps_row allocated lazily AFTER the first scores
                    # pair so the pair tiles claim the PSUM banks the QKV
                    # phase never used (avoids a transition stall)
                    ps_o = ps_row = None
                    npair = len(ts_here) // 2
                    last_pair = npair - 1

                    def emit_evrow(pi, ex, t0, t1):
                        nonlocal ps_o, ps_row
                        if ps_o is None:
                            ps_o = aps_o.tile([P, SQ], F32, tag="out",
                                              name="ps_o")
                            ps_row = aps_r.tile([P, SQ], F32, tag="row",
                                                name="ps_row")
                        for j, t in ((0, t0), (1, t1)):
                            first = pi == 0 and j == 0
                            last = pi == last_pair and j == 1
                            nc.tensor.matmul(
                                ps_o[:], v_sb[:, t, h * P:(h + 1) * P],
                                ex[:, j, :], start=first, stop=last)
                            nc.tensor.matmul(
                                ps_row[:], ones_sq[:], ex[:, j, :],
                                start=first, stop=last)

                    pend = None
                    for pi in range(npair):
                        t0, t1 = ts_here[2 * pi], ts_here[2 * pi + 1]
                        ps_s = aps_s.tile([P, 2, SQ], F32, tag="scores",
                                          name="ps_s")
                        masked = block_cls[(t0, b)] == "mask"
                        assert masked == (block_cls[(t1, b)] == "mask")
                        if masked:
                            # preload the additive mask into both PSUM
                            # banks; the scores matmuls accumulate onto it
                            nc.vector.tensor_copy(
                                ps_s[:], mp_tiles[(t0, b)])
                        for j, t in ((0, t0), (1, t1)):
                            nc.tensor.matmul(
                                ps_s[:, j, :],
                                qk_all[:, hpc + h, t * P:(t + 1) * P],
                                q_sl, start=not masked, stop=True,
                                skip_group_check=masked)
                        ex = epool.tile([P, 2, SQ], MD, tag="exp",
                                        name="ex")
                        nc.scalar.activation(
                            ex[:], ps_s[:],
                            mybir.ActivationFunctionType.Exp)
                        if pend is not None:
                            emit_evrow(*pend)
                        pend = (pi, ex, t0, t1)
                    emit_evrow(*pend)

                    recip = rpool.tile([P, SQ], F32, tag="recip",
                                       name="recip")
                    nc.vector.reciprocal_approx_fast(recip[:], ps_row[:])
                    ob = opool.tile([P, SQ], MD, tag="ob", name="ob")
                    nc.vector.tensor_mul(ob[:], ps_o[:], recip[:])
                    nc.scalar.dma_start(
                        gat_b[b][h // 2].ap()[(h % 2) * P:
                                              (h % 2 + 1) * P, :], ob[:])

                for b in range(n_sq):
                    ts_here = [t for t in range(n_st) if (t, b) in block_cls]
                    assert ts_here and len(ts_here) % 2 == 0
                    for h in range(hpc):
                        head_attention(h, b, ts_here)
                        if h % 2 == 1:
                            pp = h // 2
                            nc.gpsimd.collective_compute(
                                "AllGather", mybir.AluOpType.bypass,
                                replica_groups=[list(range(N_CORES))],
                                ins=[gat_b[b][pp].ap().opt()],
                                outs=[ct_b[b][pp].ap().opt()])
                        if b == 0 and h == 0:
                            # wo arrives while block 0's attention runs,
                            # off the startup critical path
                            nc.gpsimd.dma_start(wo_sb[:], woT_t[:])
                        if b > 1:
                            # o_proj chains lag their AllGather by two
                            # blocks: tolerant of cross-rank skew in the
                            # collective. ct is fetched from shared DRAM
                            # only now, keeping the SBUF window flat.
                            if h == 0:
                                prefetch_ct(b - 2, 0)
                                prefetch_ct(b - 2, 1)
                            else:
                                for ch in (2 * h - 2, 2 * h - 1):
                                    st, pp = ch % 4, ch // 4
                                    emit_oproj_half(b - 2, st, pp)
                    if b > 1:
                        for ch in (6, 7):
                            st, pp = ch % 4, ch // 4
                            emit_oproj_half(b - 2, st, pp)
                # tail: o_proj of the last two blocks (their collectives
                # overlap the preceding chains)
                b2, b3 = n_sq - 2, n_sq - 1
                prefetch_ct(b2, 0)
                prefetch_ct(b2, 1)
                for ch in range(4):
                    emit_oproj_half(b2, ch % 4, ch // 4)
                prefetch_ct(b3, 0)
                for ch in range(4, 8):
                    emit_oproj_half(b2, ch % 4, ch // 4)
                prefetch_ct(b3, 1)
                for ch in range(8):
                    emit_oproj_half(b3, ch % 4, ch // 4)

    nc.compile()
    return nc


def _classify_blocks(maskT_np, S):
    """Classify each [128, SQ] scoresT block of the (transposed) mask."""
    cls = {}
    for t in range(S // P):
        rows = maskT_np[t * P:(t + 1) * P]
        for b in range(S // SQ):
            blk = rows[:, b * SQ:(b + 1) * SQ]
            if np.all(blk <= -1e30):
                continue                      # fully masked: skip compute
            if np.all(blk == 0.0):
                cls[(t, b)] = "plain"
            else:
                cls[(t, b)] = "mask"
    return cls


def make_in_maps(hidden_states, attention_mask, w_pack, w_o):
    B, S, H = hidden_states.shape
    hpc = NUM_HEADS // N_CORES
    dpc = hpc * HEAD_DIM
    np_md = mybir.dt.np(_mm_dtype(MM_MODE))
    xT = np.ascontiguousarray(hidden_states[0].T).astype(np_md)
    maskT_np = np.ascontiguousarray(
        np.broadcast_to(attention_mask, (1, 1, S, S))[0, 0].T,
        dtype=np.float32)
    # w_o rows permuted to match the head-pair AllGather layout:
    # [pp][rank][head-in-pair] blocks of 128
    # mask ships as fp16 (clipped to its finite min): any value under
    # ~-100 zeroes the exp, and halving the bytes halves the PSUM preload
    # cost on the DVE
    mask16 = np.clip(maskT_np, np.finfo(np.float16).min,
                     np.finfo(np.float16).max).astype(np.float16)
    perm = np.concatenate(
        [np.arange(128 * (4 * r + 2 * pp + hh),
                   128 * (4 * r + 2 * pp + hh) + 128)
         for pp in (0, 1) for r in range(N_CORES) for hh in (0, 1)])
    in_maps = []
    for c in range(N_CORES):
        sl = slice(c * dpc, (c + 1) * dpc)
        wqk_c = np.concatenate(
            [w_pack[0 * H:1 * H][sl], w_pack[1 * H:2 * H][sl]], axis=0)
        woT_c = np.ascontiguousarray(w_o[sl].T)[perm]
        in_maps.append({
            "xT": xT,
            "wqkT": np.ascontiguousarray(wqk_c.T).astype(np_md),
            "wvT": np.ascontiguousarray(w_pack[2 * H:3 * H][sl].T
                                        ).astype(np_md),
            "maskT": mask16,
            "woT": np.ascontiguousarray(woT_c).astype(np_md),
        })
    return in_maps, maskT_np


def kernel(hidden_states, attention_mask, w_pack, w_o):
    B, S, H = hidden_states.shape
    assert B == 1 and H == NUM_HEADS * HEAD_DIM
    assert S % (2 * SQ) == 0

    in_maps, maskT_np = make_in_maps(hidden_states, attention_mask,
                                     w_pack, w_o)
    block_cls = _classify_blocks(maskT_np, S)

    key = (S, H, tuple(sorted(block_cls.items())), MM_MODE)
    if key not in _CACHE:
        _CACHE[key] = build(S, H, block_cls, MM_MODE)
    nc = _CACHE[key]

    res = run_bass_kernel_spmd(nc, in_maps, core_ids=list(range(N_CORES)))
    out = np.concatenate(
        [res.results[c]["out_cols"] for c in range(N_CORES)], axis=1)
    return out.reshape(1, S, H).astype(np.float32)
